# revision 1
# baseline (speedup 1.0000x reference)
"""BatchSRU Trainium2 kernel (nn_BatchSRU_27556510171508) — v3.

Full inputs: x (2048, 8, 128, 16) f32, W (16, 128, 384), b (16, 256).
Sharding: data-parallel over the inner batch B=8 -> one batch row per
NeuronCore (zero cross-core communication); W/b replicated.

Pipeline per core (HW-tuned; ~235us/iter vs 362us baseline):
  - host hands each core x as (L, NB, D) so every device access pattern
    is contiguous (strided PE moving-operand reads are ~4x slower on HW)
  - per instance: PE transposes x -> PSUM (shared with the x_tilde bank),
    ACT copies to bf16 SBUF, 3 bf16 matmuls (W stationary, x^T moving),
    ACT sigmoids with fused per-partition bias
  - fused g' = (f-1)*x_tilde via scalar_tensor_tensor; per-instance
    tensor_tensor_scan with carry as the initial state (state fp32
    internally): c = f*state - g'
  - highway t = c - x, u = r*t on DVE into fresh tiles (bf16 packed
    mode; GpSimd runs these ~3x slower on HW and adds cross-engine
    latency); the per-instance carry save rides on ACT
  - out-transpose + h = u^T + x fused onto the x tiles; back pieces are
    deferred via a cross-chunk queue so no engine waits on the
    scan->highway tail; per-subtile output DMA fires as soon as the last
    group's piece lands
"""

import numpy as np
from contextlib import ExitStack

import concourse.bacc as bacc
import concourse.tile as tile
from concourse import mybir
from concourse.masks import make_identity

F32 = mybir.dt.float32
BF16 = mybir.dt.bfloat16
AL = mybir.AluOpType
AF = mybir.ActivationFunctionType

L, B, D, NB = 2048, 8, 128, 16
LC = 512                 # l-chunk
NCH = L // LC            # 4 chunks
QNB = 4                  # instances per scan group
NQ = NB // QNB           # 4 groups
NLS = LC // 128          # 4 l-subtiles per chunk

N_CORES = 8


def _build(repeat: int = 1):
    nc = bacc.Bacc("TRN2")
    x = nc.dram_tensor("x", [L, NB, D], F32, kind="ExternalInput")
    w = nc.dram_tensor("w", [NB, D, 3 * D], F32, kind="ExternalInput")
    bb = nc.dram_tensor("bb", [NB, 2 * D], F32, kind="ExternalInput")
    out = nc.dram_tensor("out", [L, NB, D], F32, kind="ExternalOutput")

    with tile.TileContext(nc) as tc, ExitStack() as ctx:
        const = ctx.enter_context(tc.tile_pool(name="const", bufs=1))

        ident = const.tile([128, 128], F32)
        make_identity(nc, ident)
        identb = const.tile([128, 128], BF16)
        make_identity(nc, identb)
        wr = const.tile([128, NB, 3 * D], BF16)
        bsb = const.tile([128, NB, 2], F32)
        nc.scalar.dma_start(out=bsb, in_=bb.rearrange("n (g d) -> d n g", d=128))
        carry = const.tile([128, NB], BF16)
        nc.vector.memset(carry, 0.0)

        # W: DMA as f32 then round to f32r (the verifier requires a
        # rounding op before an f32r matmul consumes it). Four pieces on
        # the ACT ring + DVE so the x-chunk loads on SP interleave with
        # them and the first matmul only waits on the first piece.
        with tc.tile_pool(name="wtmp_pool", bufs=1) as wtmp_pool:
            wtmp = wtmp_pool.tile([128, NB, 3 * D], F32)
            for wi in range(4):
                sl = slice(wi * 4, (wi + 1) * 4)
                nc.scalar.dma_start(out=wtmp[:, sl], in_=w.transpose([1, 0, 2])[:, sl])
                nc.vector.tensor_copy(wr[:, sl], wtmp[:, sl])

        xpool = ctx.enter_context(tc.tile_pool(name="xpool", bufs=2))
        sb = ctx.enter_context(tc.tile_pool(name="sb", bufs=2))
        pu = ctx.enter_context(tc.tile_pool(name="pu", bufs=2, space="PSUM"))
        ph = ctx.enter_context(tc.tile_pool(name="ph", bufs=2, space="PSUM"))

        import contextlib

        # back pieces (out-transpose + h-add + final out-DMA) are deferred
        # across group AND chunk boundaries via this queue so no engine
        # ever waits on the scan->highway tail of the current group
        pending = []  # (ready_gidx, rw_tile, qq, ls, xts_tile, lc)
        dma_left = {}  # id(xts_tile) -> (remaining piece count, lc, ls)

        def emit_back_piece(rw, qq, ls, xts_t, plc):
            # out-transpose u = r*(c-x) for one l-subtile and fuse
            # h = u^T + x onto the x tile; after the last group's piece
            # for this subtile, emit its output DMA
            hps = ph.tile([128, QNB * 128], BF16, tag="ph", name="hps")
            for j in range(QNB):
                nc.tensor.transpose(
                    hps[:, j * 128 : (j + 1) * 128],
                    rw[:, j, ls * 128 : ls * 128 + 128],
                    identb,
                )
            xv = xts_t[:, qq * QNB * D : (qq + 1) * QNB * D]
            nc.vector.tensor_tensor(xv, hps, xv, AL.add)
            left, dlc, dls = dma_left[id(xts_t)]
            left -= 1
            dma_left[id(xts_t)] = (left, dlc, dls)
            if left == 0:
                l0 = dlc * LC + dls * 128
                nc.sync.dma_start(
                    out=out[l0 : l0 + 128].rearrange("l n d -> l (n d)"),
                    in_=xts_t,
                )

        def drain_pending(gidx, lag=6):
            if pending and gidx >= pending[0][0] + lag:
                ready, rw, qq, ls, xts_t, plc = pending.pop(0)
                emit_back_piece(rw, qq, ls, xts_t, plc)

        loop_cm = tc.For_i(0, repeat) if repeat > 1 else contextlib.nullcontext()
        with loop_cm:
         for lc in range(NCH):
            xts = []
            for ls in range(NLS):
                xt_in = xpool.tile([128, D * NB], F32, tag=f"X{ls}")
                l0 = lc * LC + ls * 128
                nc.sync.dma_start(
                    out=xt_in, in_=x[l0 : l0 + 128].rearrange("l n d -> l (n d)")
                )
                xts.append(xt_in)
                dma_left[id(xt_in)] = (NQ, lc, ls)

            # per-group SBUF tiles, 2 groups in flight
            def gtiles(q):
                s = q % 2
                xTw = sb.tile([128, QNB, LC], BF16, tag=f"xT{s}", name=f"xT{s}")
                fw = sb.tile([128, QNB, LC], BF16, tag=f"f{s}", name=f"f{s}")
                rw = sb.tile([128, QNB, LC], BF16, tag=f"r{s}", name=f"r{s}")
                gw = sb.tile([128, QNB, LC], BF16, tag=f"g{s}", name=f"g{s}")
                cw = sb.tile([128, QNB, LC], BF16, tag=f"c{s}", name=f"c{s}")
                tw = sb.tile([128, QNB, LC], BF16, tag=f"t{s}", name=f"t{s}")
                uw = sb.tile([128, QNB, LC], BF16, tag=f"u{s}", name=f"u{s}")
                return xTw, fw, rw, gw, cw, tw, uw

            def in_transpose(i, pui, xTw):
                # 4 l-subtiles of instance nb -> psum bank 0 of pu tile
                j = i % QNB
                for ls in range(NLS):
                    xg = xts[ls][:, i * D : (i + 1) * D]
                    nc.tensor.transpose(
                        pui[:, 0, ls * 128 : (ls + 1) * 128], xg, ident
                    )
                # rounding copy psum -> bf16 SBUF (ACT)
                nc.scalar.copy(xTw[:, j], pui[:, 0])

            grp = {}  # q -> group SBUF tiles

            pu_i = [None] * (NB + 1)
            # prime: in-transpose for instance 0
            grp[0] = gtiles(0)
            pu_i[0] = pu.tile([128, 3, LC], F32, tag="pu", name="pu")
            in_transpose(0, pu_i[0], grp[0][0])

            for i in range(NB):
                q, j = i // QNB, i % QNB
                gidx = lc * NB + i
                xTw, fw, rw, gw, cw, tw, uw = grp[q]

                # next instance's transpose ahead of this one's matmuls
                if i + 1 < NB:
                    qn = (i + 1) // QNB
                    if (i + 1) % QNB == 0:
                        grp[qn] = gtiles(qn)
                    pu_i[i + 1] = pu.tile([128, 3, LC], F32, tag="pu", name="pu")
                    in_transpose(i + 1, pu_i[i + 1], grp[qn][0])

                pui = pu_i[i]
                # matmuls: x_tilde overwrites the transpose bank after the copy
                nc.tensor.matmul(
                    pui[:, 0], wr[:, i, 0:128], xTw[:, j], start=True, stop=True
                )
                nc.tensor.matmul(
                    pui[:, 1], wr[:, i, 128:256], xTw[:, j], start=True, stop=True
                )
                nc.tensor.matmul(
                    pui[:, 2], wr[:, i, 256:384], xTw[:, j], start=True, stop=True
                )
                # gates (ACT): f = sigmoid(f_pre + bf), r = sigmoid(r_pre + br)
                nc.scalar.activation(
                    fw[:, j], pui[:, 1], AF.Sigmoid, bias=bsb[:, i, 0:1], scale=1.0
                )
                nc.scalar.activation(
                    rw[:, j], pui[:, 2], AF.Sigmoid, bias=bsb[:, i, 1:2], scale=1.0
                )
                # g' = (f - 1) * x_tilde  (DVE, fused)
                nc.vector.scalar_tensor_tensor(
                    gw[:, j], fw[:, j], -1.0, pui[:, 0], AL.add, AL.mult
                )
                # per-instance scan with carry as the initial state:
                # state = f*state - g'  (scan state stays fp32 internally)
                nc.vector.tensor_tensor_scan(
                    cw[:, j],
                    fw[:, j],
                    gw[:, j],
                    carry[:, i : i + 1],
                    op0=AL.mult,
                    op1=AL.subtract,
                )
                # save carry before c is overwritten, then highway
                # precompute t = c - x, u = r * t. All on DVE: GpSimd runs
                # these ~3x slower on HW and adds cross-engine latency
                # right in the scan->out-transpose chain.
                nc.scalar.copy(carry[:, i : i + 1], cw[:, j, LC - 1 : LC])
                nc.vector.tensor_tensor(tw[:, j], cw[:, j], xTw[:, j], AL.subtract)
                nc.vector.tensor_tensor(uw[:, j], rw[:, j], tw[:, j], AL.mult)
                if j == QNB - 1:
                    for ls in range(NLS):
                        pending.append((gidx, uw, q, ls, xts[ls], lc))
                drain_pending(gidx)

         # flush remaining back pieces + final out-DMAs
         while pending:
            drain_pending(1 << 30)

    nc.finalize()
    return nc


_NC_CACHE = None


def _get_nc():
    global _NC_CACHE
    if _NC_CACHE is None:
        _NC_CACHE = _build()
    return _NC_CACHE


def make_in_maps(x, W, b):
    # per-core layout (L, NB, D): every on-device access pattern is then
    # contiguous (strided PE moving-operand reads are ~4x slower on HW)
    return [
        dict(x=np.ascontiguousarray(x[:, i].transpose(0, 2, 1)), w=W, bb=b)
        for i in range(N_CORES)
    ]


def assemble(outs):
    # outs: per-core (L, NB, D) -> full (L, B, D, NB)
    return np.stack([o.transpose(0, 2, 1) for o in outs], axis=1)


def kernel(x: np.ndarray, W: np.ndarray, b: np.ndarray) -> np.ndarray:
    assert x.shape == (L, B, D, NB) and W.shape == (NB, D, 3 * D)
    from concourse.bass_utils import run_bass_kernel_spmd

    nc = _get_nc()
    x = np.asarray(x, dtype=np.float32)
    W = np.asarray(W, dtype=np.float32)
    b = np.asarray(b, dtype=np.float32)
    in_maps = make_in_maps(x, W, b)
    results = run_bass_kernel_spmd(nc, in_maps, core_ids=list(range(N_CORES))).results
    return assemble([results[i]["out"] for i in range(N_CORES)])

